# revision 12
# baseline (speedup 1.0000x reference)
"""Trainium2 Bass kernel for nn_Atomic_node_only_lstm (GNN message passing + BiLSTM + MLP).

v3: pair-width (FD=1024) elementwise ops, ones-row bias folds (merged sigmoid
ACTs), DMA-broadcast attention replication (replaces RA matmuls + evacs),
quantity-pair PSUM allocations, 2-lane interleave.

Data-parallel over 8 NeuronCores (batch N=65536 -> 8192/core).
Pairs of NB=512 batch columns processed as FD=1024 tiles; matmuls N=512.

Device layout per pair (free dim 1024 = 2 x 512 batch cols), s-step groups
g=0,1,2 -> slots (s0,s1),(s2,s3),(s4,zero):
  hg[g]  [97, 1024] bf16: rows 0:96 h (j*48+k*12+d), row 96 = ones (bias row)
  att32[g] [32, 1024] bf16: rows j*16+n*4+w = attmat[n,w] of slot j
  arb[g][n] [96, 1024] bf16: att[n,w] replicated over d (DMA broadcast)
  Q_n = arb_n * hg[0:96]
  hbar via SR matmuls (sum over w); hb [128,1024] = [hbar(96) | att(32)]
  gates: r,z: K=97 H-part (h+ones, const biases folded) + K=128 HB-part
  (hbar + att rows folding s_n*(W_ih@msg_b)); inn: K=128 HB; hn: K=97 H.
  sigma(r),sigma(z) bias-free ACTs; tanh ACT carries b_inn.

LSTM: XL [97,1024] = [x_fw | x_bw | ones] (gate biases folded via ones);
  HL [97,1024] = [h_fw | h_bw | ones] (ones for MLP en1 bias fold);
  psum allocs per step: i,f,g,o quantity-pairs [96,1024].
MLP: We1 [97,48], e1 [49,1024] (ones row 48), We2 [49,36], e2 [37,1024],
  We3 [37,6]; all biases folded; final copy psum->sbuf f32.
"""
import numpy as np
import ml_dtypes
from contextlib import ExitStack

N, S, K, D, H = 65536, 5, 4, 12, 48
NCORES = 8
NCORE = N // NCORES          # 8192 batch elements per core
NB = 512                     # batch columns per matmul
PW = 2 * NB                  # pair width for elementwise ops
NPAIRS = NCORE // PW         # 8
NGROUPS = 3
NLANES = 2                   # concurrently-emitted pairs

BF = ml_dtypes.bfloat16
ARB_VIA_DMA = True           # broadcast-DMA att replication (else RA matmuls)
BF16_PSUM = False            # bf16 matmul outputs rejected by bass (fp32 only)


# ----------------------------------------------------------------------------
# host-side weight construction (numpy, all tiny)
# ----------------------------------------------------------------------------
def build_weights(msg_w, msg_b, gru_w_ih, gru_w_hh, gru_b_ih, gru_b_hh,
                  lstm_w_ih_fw, lstm_w_hh_fw, lstm_b_ih_fw, lstm_b_hh_fw,
                  lstm_w_ih_bw, lstm_w_hh_bw, lstm_b_ih_bw, lstm_b_hh_bw,
                  en1_w, en1_b, en2_w, en2_b, en3_w, en3_b):
    out = {}
    A_r = gru_w_ih[0:12] @ msg_w
    A_z = gru_w_ih[12:24] @ msg_w
    A_n = gru_w_ih[24:36] @ msg_w
    bi_r = gru_w_ih[0:12] @ msg_b
    bi_z = gru_w_ih[12:24] @ msg_b
    bi_n = gru_w_ih[24:36] @ msg_b

    # RA_n lhsT [32, 96] fallback path (row-quadrant placement on device).
    for n in range(4):
        R = np.zeros((32, 96), np.float32)
        for j in range(2):
            for w in range(4):
                for d in range(12):
                    R[j * 16 + n * 4 + w, j * 48 + w * 12 + d] = 1.0
        out[f"RA{n}"] = R

    # SR_n lhsT [96, 96]: contract Q_n -> hbar contribution of node n.
    for n in range(4):
        Sm = np.zeros((96, 96), np.float32)
        for j in range(2):
            for w in range(4):
                for d in range(12):
                    Sm[j * 48 + w * 12 + d, j * 48 + n * 12 + d] = 1.0
        out[f"SR{n}"] = Sm

    # GRU H-part lhsTs [97, 96]: rows 0:96 block-diag W_hh gate block,
    # row 96 = per-column constant bias (via the ones row of hg).
    def gate_H97(whh_blk, bconst):
        Wm = np.zeros((97, 96), np.float32)
        for j in range(2):
            for k in range(4):
                for d in range(12):
                    c = j * 48 + k * 12 + d
                    if whh_blk is not None:
                        for dp in range(12):
                            Wm[j * 48 + k * 12 + dp, c] = whh_blk[d, dp]
                    if bconst is not None:
                        Wm[96, c] = bconst[d]
        return Wm

    # HB-part lhsTs [128, 96]: rows 0:96 block-diag A-gate block on hbar,
    # rows 96:128 att rows folding s_n * (W_ih@msg_b).
    def gate_HB128(A_blk, bi_blk):
        Wm = np.zeros((128, 96), np.float32)
        for j in range(2):
            for k in range(4):
                for d in range(12):
                    c = j * 48 + k * 12 + d
                    for dp in range(12):
                        Wm[j * 48 + k * 12 + dp, c] = A_blk[d, dp]
                    if bi_blk is not None:
                        for w in range(4):
                            Wm[96 + j * 16 + k * 4 + w, c] = bi_blk[d]
        return Wm

    out["W_r_H"] = gate_H97(gru_w_hh[0:12], gru_b_ih[0:12] + gru_b_hh[0:12])
    out["W_z_H"] = gate_H97(gru_w_hh[12:24], gru_b_ih[12:24] + gru_b_hh[12:24])
    out["W_hn_H"] = gate_H97(gru_w_hh[24:36], gru_b_hh[24:36])
    out["W_r_HB"] = gate_HB128(A_r, bi_r)
    out["W_z_HB"] = gate_HB128(A_z, bi_z)
    out["W_inn_HB"] = gate_HB128(A_n, bi_n)
    out["b_inn96"] = np.tile(gru_b_ih[24:36], 8).reshape(96, 1).astype(np.float32)

    # LSTM lhsTs. X-part [97, 96]: rows j*48+k*12+d map x (jj = d*4+k),
    # row 96 = folded gate bias. HL-part [96, 96] block-diag W_hh.T.
    gates = {"I": 0, "F": 1, "G": 2, "O": 3}
    wih = {"fw": lstm_w_ih_fw, "bw": lstm_w_ih_bw}
    whh = {"fw": lstm_w_hh_fw, "bw": lstm_w_hh_bw}
    bih = {"fw": lstm_b_ih_fw, "bw": lstm_b_ih_bw}
    bhh = {"fw": lstm_b_hh_fw, "bw": lstm_b_hh_bw}
    for gname, g in gates.items():
        X = np.zeros((97, 96), np.float32)
        Hm = np.zeros((96, 96), np.float32)
        for j, dirn in enumerate(("fw", "bw")):
            wg = wih[dirn][g * 48:(g + 1) * 48, :]
            hg_ = whh[dirn][g * 48:(g + 1) * 48, :]
            for k in range(4):
                for d in range(12):
                    X[j * 48 + k * 12 + d, j * 48:(j + 1) * 48] = wg[:, d * 4 + k]
            X[96, j * 48:(j + 1) * 48] = (bih[dirn][g * 48:(g + 1) * 48]
                                          + bhh[dirn][g * 48:(g + 1) * 48])
            Hm[j * 48:(j + 1) * 48, j * 48:(j + 1) * 48] = hg_.T
        out[f"L_{gname}_X"] = X
        out[f"L_{gname}_HL"] = Hm

    # MLP with folded biases via ones rows.
    We1 = np.zeros((97, 48), np.float32)
    We1[0:48, :] = en1_w[:, 0:48].T
    We1[48:96, :] = en1_w[:, 48:96].T
    We1[96, :] = en1_b
    out["We1"] = We1
    We2 = np.zeros((49, 36), np.float32)
    We2[0:48, :] = en2_w.T
    We2[48, :] = en2_b
    out["We2"] = We2
    We3 = np.zeros((37, 6), np.float32)
    We3[0:36, :] = en3_w.T
    We3[36, :] = en3_b
    out["We3"] = We3
    return out


def prep_inputs(nodes_feature, pos, attmat):
    """feat [S, 48, N] bf16 (k-major (k,d)), att [S, 16, N] bf16,
    arbm [3, 4, 96, N] bf16 (att d-replicated: arbm[g,n,j*48+w*12+d] =
    att[2g+j, n, w]; g=2 j=1 rows zero)."""
    nf = np.concatenate([nodes_feature, pos], axis=-1)       # [N,S,K,12]
    feat = np.ascontiguousarray(nf.transpose(1, 2, 3, 0)).reshape(S, 48, N)
    att = np.ascontiguousarray(attmat.transpose(1, 2, 3, 0)).reshape(S, 16, N)
    att_r = att.reshape(S, 4, 4, N)                          # [s, n, w, N]
    arbm = np.zeros((NGROUPS, 4, 2, 4, 12, N), dtype=BF)
    for g in range(NGROUPS):
        for j in range(2 if g < 2 else 1):
            # [n, w, N] -> replicate d -> [n, w, 12, N]
            arbm[g, :, j] = np.repeat(
                att_r[2 * g + j].astype(BF)[:, :, None, :], 12, axis=2)
    arbm = arbm.reshape(NGROUPS, 4, 96, N)
    return feat.astype(BF), att.astype(BF), arbm


# ----------------------------------------------------------------------------
# device kernel builder
# ----------------------------------------------------------------------------
def split_excess_waits(nc, max_waits=1):
    import concourse.mybir as mybir
    cnt = 0
    for f in nc.m.functions:
        for bb in f.blocks:
            insts = bb.instructions
            new = []
            changed = False
            for inst in insts:
                si = inst.sync_info
                waits = list(si.on_wait) if si and si.on_wait else []
                if len(waits) > max_waits:
                    changed = True
                    k = 0
                    while len(waits) - k > max_waits:
                        chunk = waits[k:k + max_waits]
                        k += max_waits
                        cnt += 1
                        nop = mybir.InstNoOp(name=f"waitsplit-{cnt}", ins=[], outs=[])
                        nop.engine = inst.engine
                        nop.sync_info = mybir.SyncInfo(on_wait=chunk, on_update=[])
                        new.append(nop)
                    inst.sync_info = mybir.SyncInfo(
                        on_wait=waits[k:],
                        on_update=list(si.on_update) if si.on_update else [])
                new.append(inst)
            if changed:
                bb.instructions = new
    return cnt


WEIGHT_SPECS = None  # filled in build_nc


def build_nc():
    import concourse.bass as bass
    import concourse.tile as tile
    from concourse import mybir

    f32 = mybir.dt.float32
    bf16 = mybir.dt.bfloat16
    AF = mybir.ActivationFunctionType
    ALU = mybir.AluOpType
    PSDT = bf16 if BF16_PSUM else f32

    nc = bass.Bass("TRN2")

    feat_d = nc.dram_tensor("feat", [S, 48, NCORE], bf16, kind="ExternalInput")
    att_d = nc.dram_tensor("att", [S, 16, NCORE], bf16, kind="ExternalInput")
    arbm_d = nc.dram_tensor("arbm", [NGROUPS, 4, 96, NCORE], bf16,
                            kind="ExternalInput")

    wspecs = []
    for n in range(4):
        wspecs.append((f"SR{n}", (96, 96), bf16))
        if not ARB_VIA_DMA:
            wspecs.append((f"RA{n}", (32, 96), bf16))
    wspecs += [("W_r_H", (97, 96), bf16), ("W_z_H", (97, 96), bf16),
               ("W_hn_H", (97, 96), bf16),
               ("W_r_HB", (128, 96), bf16), ("W_z_HB", (128, 96), bf16),
               ("W_inn_HB", (128, 96), bf16),
               ("b_inn96", (96, 1), f32)]
    for gname in "IFGO":
        wspecs.append((f"L_{gname}_X", (97, 96), bf16))
        wspecs.append((f"L_{gname}_HL", (96, 96), bf16))
    wspecs += [("We1", (97, 48), bf16), ("We2", (49, 36), bf16),
               ("We3", (37, 6), bf16)]

    wnames = {}
    for nm, shp, dt in wspecs:
        wnames[nm] = nc.dram_tensor(nm, list(shp), dt, kind="ExternalInput")
    out_d = nc.dram_tensor("out", [6, NCORE], f32, kind="ExternalOutput")
    RA_QUAD = {0: 0, 1: 32, 2: 64, 3: 96}

    global WEIGHT_SPECS
    WEIGHT_SPECS = wspecs

    with tile.TileContext(nc) as tc:
        with ExitStack() as ctx:
            wpool = ctx.enter_context(tc.tile_pool(name="weights", bufs=1))
            wt = {}
            for nm, shp, dt in wspecs:
                if nm.startswith("RA"):
                    q = RA_QUAD[int(nm[2])]
                    t = wpool.tile([128, shp[1]], dt, tag=f"w_{nm}")
                    nc.sync.dma_start(t[q:q + 32, :], wnames[nm][:])
                    wt[nm] = t
                else:
                    t = wpool.tile([shp[0], shp[1]], dt, tag=f"w_{nm}")
                    nc.sync.dma_start(t[:], wnames[nm][:])
                    wt[nm] = t

            # persistent per-lane state tiles (ones rows / zero rows set once)
            stp = ctx.enter_context(tc.tile_pool(name="state", bufs=1))
            HG, ATT, ARB, XL, HL, CC, E1, E2 = {}, {}, {}, {}, {}, {}, {}, {}
            for ln in range(NLANES):
                for g in range(NGROUPS):
                    hg = stp.tile([97, PW], bf16, tag=f"hg{ln}_{g}", name=f"hg{ln}_{g}")
                    nc.vector.memset(hg[96:97, :], 1.0)
                    if g == 2:
                        nc.vector.memset(hg[48:96, :], 0.0)
                    HG[(ln, g)] = hg
                    at = stp.tile([32, PW], bf16, tag=f"att{ln}_{g}", name=f"att{ln}_{g}")
                    if g == 2:
                        nc.vector.memset(at[16:32, :], 0.0)
                    ATT[(ln, g)] = at
                    for n in range(4):
                        ab = stp.tile([96, PW], bf16, tag=f"arb{ln}_{g}_{n}",
                                      name=f"arb{ln}_{g}_{n}")
                        if g == 2:
                            nc.vector.memset(ab[48:96, :], 0.0)
                        ARB[(ln, g, n)] = ab
                for kx in range(2):
                    xl = stp.tile([97, PW], bf16, tag=f"xl{ln}_{kx}", name=f"xl{ln}_{kx}")
                    nc.vector.memset(xl[96:97, :], 1.0)
                    XL[(ln, kx)] = xl
                hl = stp.tile([97, PW], bf16, tag=f"hl{ln}", name=f"hl{ln}")
                nc.vector.memset(hl[96:97, :], 1.0)
                HL[ln] = hl
                CC[ln] = stp.tile([96, PW], bf16, tag=f"cc{ln}", name=f"cc{ln}")
                e1 = stp.tile([49, PW], bf16, tag=f"e1{ln}", name=f"e1{ln}")
                nc.vector.memset(e1[48:49, :], 1.0)
                E1[ln] = e1
                E2[ln] = stp.tile([37, PW], bf16, tag=f"e2{ln}", name=f"e2{ln}")

            sbp = ctx.enter_context(tc.tile_pool(name="work", bufs=2))
            psp = ctx.enter_context(tc.tile_pool(name="ps", bufs=2, space="PSUM"))

            def ps_alloc(tag, name, dt=PSDT):
                return psp.tile([96, PW], dt, tag=tag, name=name)

            def emit_pair(ip, ln):
                c0 = ip * PW
                uid = f"p{ip}"
                # ---- loads ----
                for g in range(NGROUPS):
                    hg = HG[(ln, g)]
                    at = ATT[(ln, g)]
                    nc.sync.dma_start(hg[0:48, :], feat_d[2 * g, :, c0:c0 + PW])
                    nc.sync.dma_start(at[0:16, :], att_d[2 * g, :, c0:c0 + PW])
                    if g < 2:
                        nc.sync.dma_start(hg[48:96, :],
                                          feat_d[2 * g + 1, :, c0:c0 + PW])
                        nc.sync.dma_start(at[16:32, :],
                                          att_d[2 * g + 1, :, c0:c0 + PW])
                    # arb replication: row (j,w,d) <- att32 row (j, n, w)
                    for n in range(4):
                        arb = ARB[(ln, g, n)]
                        if ARB_VIA_DMA:
                            rows = 96 if g < 2 else 48
                            nc.sync.dma_start(arb[0:rows, :],
                                              arbm_d[g, n, 0:rows, c0:c0 + PW])

                yield  # loads emitted

                # ---- 2 GRU passes ----
                for pas in range(2):
                    for g in range(NGROUPS):
                        hg = HG[(ln, g)]
                        # Q_n = arb_n * h  (pair-wide)
                        Q = []
                        for n in range(4):
                            q = sbp.tile([96, PW], bf16, tag=f"Q{n}")
                            eng = nc.gpsimd if n == 3 else nc.vector
                            eng.tensor_tensor(q[:], ARB[(ln, g, n)][:],
                                              hg[0:96, :], ALU.mult)
                            Q.append(q)
                        # hbar = sum_n SR_n @ Q_n  (per NB half)
                        ps_hb = ps_alloc("pnh", f"hb_{uid}_{g}_{pas}")
                        for h2 in range(2):
                            sl = ps_hb[:, h2 * NB:(h2 + 1) * NB]
                            for n in range(4):
                                nc.tensor.matmul(
                                    sl, wt[f"SR{n}"][:],
                                    Q[n][:, h2 * NB:(h2 + 1) * NB],
                                    start=(n == 0), stop=(n == 3))
                        hb = sbp.tile([128, PW], bf16, tag="HBs")
                        nc.vector.tensor_copy(hb[0:96, :], ps_hb[:])
                        nc.sync.dma_start(hb[96:128, :], ATT[(ln, g)][:])

                        # gates (quantity-pair psums)
                        ps_r = ps_alloc("prz", f"r_{uid}_{g}_{pas}")
                        ps_z = ps_alloc("prz", f"z_{uid}_{g}_{pas}")
                        ps_in = ps_alloc("pnh", f"in_{uid}_{g}_{pas}")
                        ps_hn = ps_alloc("pnh", f"hn_{uid}_{g}_{pas}")
                        for h2 in range(2):
                            cs = slice(h2 * NB, (h2 + 1) * NB)
                            nc.tensor.matmul(ps_r[:, cs], wt["W_r_H"][:],
                                             hg[0:97, cs], start=True, stop=False)
                            nc.tensor.matmul(ps_r[:, cs], wt["W_r_HB"][:],
                                             hb[0:128, cs], start=False, stop=True)
                            nc.tensor.matmul(ps_z[:, cs], wt["W_z_H"][:],
                                             hg[0:97, cs], start=True, stop=False)
                            nc.tensor.matmul(ps_z[:, cs], wt["W_z_HB"][:],
                                             hb[0:128, cs], start=False, stop=True)
                            nc.tensor.matmul(ps_in[:, cs], wt["W_inn_HB"][:],
                                             hb[0:128, cs], start=True, stop=True)
                            nc.tensor.matmul(ps_hn[:, cs], wt["W_hn_H"][:],
                                             hg[0:97, cs], start=True, stop=True)

                        yield  # gates emitted; let other lane feed PE
                        sr = sbp.tile([96, PW], bf16, tag="SR_")
                        sz = sbp.tile([96, PW], bf16, tag="SZ_")
                        nc.scalar.activation(sr[:], ps_r[:], AF.Sigmoid)
                        nc.scalar.activation(sz[:], ps_z[:], AF.Sigmoid)
                        t1 = sbp.tile([96, PW], bf16, tag="t1")
                        nc.vector.tensor_tensor(t1[:], ps_hn[:], sr[:], ALU.mult)
                        u = sbp.tile([96, PW], bf16, tag="u")
                        nc.vector.tensor_tensor(u[:], ps_in[:], t1[:], ALU.add)
                        tn = sbp.tile([96, PW], bf16, tag="tn")
                        nc.scalar.activation(tn[:], u[:], AF.Tanh,
                                             bias=wt["b_inn96"][:, 0:1])
                        v = sbp.tile([96, PW], bf16, tag="v")
                        nc.vector.tensor_tensor(v[:], hg[0:96, :], tn[:],
                                                ALU.subtract)
                        w2 = sbp.tile([96, PW], bf16, tag="w2")
                        nc.vector.tensor_tensor(w2[:], sz[:], v[:], ALU.mult)
                        rows = slice(0, 48) if g == 2 else slice(0, 96)
                        nc.vector.tensor_tensor(hg[rows, :], tn[rows, :],
                                                w2[rows, :], ALU.add)
                        yield  # (pass, group) emitted

                # ---- BiLSTM ----
                hl = HL[ln]
                cc = CC[ln]
                for t in range(S):
                    sf_, sb_ = t, 4 - t
                    xl = XL[(ln, t % 2)]
                    nc.sync.dma_start(
                        xl[0:48, :],
                        HG[(ln, sf_ // 2)][(sf_ % 2) * 48:(sf_ % 2) * 48 + 48, :])
                    nc.sync.dma_start(
                        xl[48:96, :],
                        HG[(ln, sb_ // 2)][(sb_ % 2) * 48:(sb_ % 2) * 48 + 48, :])
                    ps_g = {}
                    for gname, tag in (("I", "prz"), ("F", "prz"),
                                       ("G", "pnh"), ("O", "pnh")):
                        if t == 0 and gname == "F":
                            continue
                        ps = ps_alloc(tag, f"L{gname}_{uid}_{t}")
                        ps_g[gname] = ps
                        for h2 in range(2):
                            cs = slice(h2 * NB, (h2 + 1) * NB)
                            nc.tensor.matmul(ps[:, cs], wt[f"L_{gname}_X"][:],
                                             xl[0:97, cs],
                                             start=True, stop=(t == 0))
                            if t > 0:
                                nc.tensor.matmul(ps[:, cs],
                                                 wt[f"L_{gname}_HL"][:],
                                                 hl[0:96, cs],
                                                 start=False, stop=True)
                    si = sbp.tile([96, PW], bf16, tag="si")
                    nc.scalar.activation(si[:], ps_g["I"][:], AF.Sigmoid)
                    tg = sbp.tile([96, PW], bf16, tag="tg")
                    nc.scalar.activation(tg[:], ps_g["G"][:], AF.Tanh)
                    t1l = sbp.tile([96, PW], bf16, tag="t1l")
                    nc.vector.tensor_tensor(t1l[:], si[:], tg[:], ALU.mult)
                    if t == 0:
                        nc.vector.tensor_copy(cc[:], t1l[:])
                    else:
                        sf2 = sbp.tile([96, PW], bf16, tag="sf2")
                        nc.scalar.activation(sf2[:], ps_g["F"][:], AF.Sigmoid)
                        t2l = sbp.tile([96, PW], bf16, tag="t2l")
                        nc.gpsimd.tensor_tensor(t2l[:], sf2[:], cc[:], ALU.mult)
                        nc.vector.tensor_tensor(cc[:], t1l[:], t2l[:], ALU.add)
                    tc2 = sbp.tile([96, PW], bf16, tag="tc2")
                    nc.scalar.activation(tc2[:], cc[:], AF.Tanh)
                    so = sbp.tile([96, PW], bf16, tag="so")
                    nc.scalar.activation(so[:], ps_g["O"][:], AF.Sigmoid)
                    nc.vector.tensor_tensor(hl[0:96, :], so[:], tc2[:], ALU.mult)
                    yield  # LSTM step emitted

                # ---- MLP ----
                psE = ps_alloc("prz", f"psE_{uid}", dt=f32)
                for h2 in range(2):
                    cs = slice(h2 * NB, (h2 + 1) * NB)
                    nc.tensor.matmul(psE[0:48, cs], wt["We1"][:], hl[0:97, cs],
                                     start=True, stop=True)
                e1 = E1[ln]
                nc.scalar.activation(e1[0:48, :], psE[0:48, :], AF.Relu)
                psE2 = ps_alloc("pnh", f"psE2_{uid}", dt=f32)
                for h2 in range(2):
                    cs = slice(h2 * NB, (h2 + 1) * NB)
                    nc.tensor.matmul(psE2[0:36, cs], wt["We2"][:], e1[0:49, cs],
                                     start=True, stop=True)
                e2 = E2[ln]
                nc.scalar.activation(e2[0:36, :], psE2[0:36, :], AF.Relu)
                psE3 = ps_alloc("prz", f"psE3_{uid}", dt=f32)
                for h2 in range(2):
                    cs = slice(h2 * NB, (h2 + 1) * NB)
                    nc.tensor.matmul(psE3[0:6, cs], wt["We3"][:], e2[0:37, cs],
                                     start=True, stop=True)
                o = sbp.tile([6, PW], f32, tag="o")
                nc.vector.tensor_copy(o[:], psE3[0:6, :])
                nc.sync.dma_start(out_d[:, c0:c0 + PW], o[:])

            # Continuous staggered pipeline: lane 0 runs pairs 0,2,4,..,
            # lane 1 runs 1,3,5,..; lane 0 is primed half a pair ahead so
            # one lane's scalar-heavy LSTM overlaps the other's vector-heavy
            # GRU. A lane starts its next pair as soon as it finishes.
            nxt = [0, 1]
            gens = [emit_pair(0, 0), None]
            nxt[0] = 2
            for _ in range(7):          # prime lane 0 ~half a pair
                next(gens[0])
            gens[1] = emit_pair(1, 1)
            live = 2
            while live:
                for i in range(NLANES):
                    g_ = gens[i]
                    if g_ is None:
                        continue
                    try:
                        next(g_)
                    except StopIteration:
                        if nxt[i] < NPAIRS:
                            gens[i] = emit_pair(nxt[i], i)
                            nxt[i] += NLANES
                        else:
                            gens[i] = None
                            live -= 1

    split_excess_waits(nc)
    return nc


_NC_CACHE = None
TRACE = False
LAST_EXEC_NS = None


def kernel(nodes_feature, pos, attmat, **w):
    global _NC_CACHE, LAST_EXEC_NS
    from concourse.bass_utils import run_bass_kernel_spmd
    import concourse.mybir as mybir

    feat, att, arbm = prep_inputs(nodes_feature, pos, attmat)
    wts = build_weights(**w)

    if _NC_CACHE is None:
        _NC_CACHE = build_nc()
    nc = _NC_CACHE

    in_maps = []
    for c in range(NCORES):
        m = {"feat": np.ascontiguousarray(feat[:, :, c * NCORE:(c + 1) * NCORE]),
             "att": np.ascontiguousarray(att[:, :, c * NCORE:(c + 1) * NCORE]),
             "arbm": np.ascontiguousarray(arbm[:, :, :, c * NCORE:(c + 1) * NCORE])}
        for nm, shp, dt in WEIGHT_SPECS:
            m[nm] = wts[nm].astype(BF) if dt == mybir.dt.bfloat16 else wts[nm].astype(np.float32)
        in_maps.append(m)

    res = run_bass_kernel_spmd(nc, in_maps, core_ids=list(range(NCORES)),
                               trace=TRACE)
    LAST_EXEC_NS = res.exec_time_ns
    outs = [res.results[c]["out"] for c in range(NCORES)]     # [6, NCORE] each
    full = np.concatenate(outs, axis=1)                        # [6, N]
    return np.ascontiguousarray(full.T).astype(np.float32)     # [N, 6]


# revision 13
# speedup vs baseline: 1.3779x; 1.3779x over previous
"""Trainium2 Bass kernel for nn_Atomic_node_only_lstm (GNN message passing + BiLSTM + MLP).

v3: pair-width (FD=1024) elementwise ops, ones-row bias folds (merged sigmoid
ACTs), DMA-broadcast attention replication (replaces RA matmuls + evacs),
quantity-pair PSUM allocations, 2-lane interleave.

Data-parallel over 8 NeuronCores (batch N=65536 -> 8192/core).
Pairs of NB=512 batch columns processed as FD=1024 tiles; matmuls N=512.

Device layout per pair (free dim 1024 = 2 x 512 batch cols), s-step groups
g=0,1,2 -> slots (s0,s1),(s2,s3),(s4,zero):
  hg[g]  [97, 1024] bf16: rows 0:96 h (j*48+k*12+d), row 96 = ones (bias row)
  att32[g] [32, 1024] bf16: rows j*16+n*4+w = attmat[n,w] of slot j
  arb[g][n] [96, 1024] bf16: att[n,w] replicated over d (DMA broadcast)
  Q_n = arb_n * hg[0:96]
  hbar via SR matmuls (sum over w); hb [128,1024] = [hbar(96) | att(32)]
  gates: r,z: K=97 H-part (h+ones, const biases folded) + K=128 HB-part
  (hbar + att rows folding s_n*(W_ih@msg_b)); inn: K=128 HB; hn: K=97 H.
  sigma(r),sigma(z) bias-free ACTs; tanh ACT carries b_inn.

LSTM: XL [97,1024] = [x_fw | x_bw | ones] (gate biases folded via ones);
  HL [97,1024] = [h_fw | h_bw | ones] (ones for MLP en1 bias fold);
  psum allocs per step: i,f,g,o quantity-pairs [96,1024].
MLP: We1 [97,48], e1 [49,1024] (ones row 48), We2 [49,36], e2 [37,1024],
  We3 [37,6]; all biases folded; final copy psum->sbuf f32.
"""
import numpy as np
import ml_dtypes
from contextlib import ExitStack

N, S, K, D, H = 65536, 5, 4, 12, 48
NCORES = 8
NCORE = N // NCORES          # 8192 batch elements per core
NB = 512                     # batch columns per matmul
PW = 2 * NB                  # pair width for elementwise ops
NPAIRS = NCORE // PW         # 8
NGROUPS = 3
NLANES = 2                   # concurrently-emitted pairs

BF = ml_dtypes.bfloat16
ARB_VIA_DMA = True           # broadcast-DMA att replication (else RA matmuls)
BF16_PSUM = False            # bf16 matmul outputs rejected by bass (fp32 only)


# ----------------------------------------------------------------------------
# host-side weight construction (numpy, all tiny)
# ----------------------------------------------------------------------------
def build_weights(msg_w, msg_b, gru_w_ih, gru_w_hh, gru_b_ih, gru_b_hh,
                  lstm_w_ih_fw, lstm_w_hh_fw, lstm_b_ih_fw, lstm_b_hh_fw,
                  lstm_w_ih_bw, lstm_w_hh_bw, lstm_b_ih_bw, lstm_b_hh_bw,
                  en1_w, en1_b, en2_w, en2_b, en3_w, en3_b):
    out = {}
    A_r = gru_w_ih[0:12] @ msg_w
    A_z = gru_w_ih[12:24] @ msg_w
    A_n = gru_w_ih[24:36] @ msg_w
    bi_r = gru_w_ih[0:12] @ msg_b
    bi_z = gru_w_ih[12:24] @ msg_b
    bi_n = gru_w_ih[24:36] @ msg_b

    # RA_n lhsT [32, 96] fallback path (row-quadrant placement on device).
    for n in range(4):
        R = np.zeros((32, 96), np.float32)
        for j in range(2):
            for w in range(4):
                for d in range(12):
                    R[j * 16 + n * 4 + w, j * 48 + w * 12 + d] = 1.0
        out[f"RA{n}"] = R

    # SR_n lhsT [96, 96]: contract Q_n -> hbar contribution of node n.
    for n in range(4):
        Sm = np.zeros((96, 96), np.float32)
        for j in range(2):
            for w in range(4):
                for d in range(12):
                    Sm[j * 48 + w * 12 + d, j * 48 + n * 12 + d] = 1.0
        out[f"SR{n}"] = Sm

    # GRU H-part lhsTs [97, 96]: rows 0:96 block-diag W_hh gate block,
    # row 96 = per-column constant bias (via the ones row of hg).
    def gate_H97(whh_blk, bconst):
        Wm = np.zeros((97, 96), np.float32)
        for j in range(2):
            for k in range(4):
                for d in range(12):
                    c = j * 48 + k * 12 + d
                    if whh_blk is not None:
                        for dp in range(12):
                            Wm[j * 48 + k * 12 + dp, c] = whh_blk[d, dp]
                    if bconst is not None:
                        Wm[96, c] = bconst[d]
        return Wm

    # HB-part lhsTs [128, 96]: rows 0:96 block-diag A-gate block on hbar,
    # rows 96:128 att rows folding s_n * (W_ih@msg_b).
    def gate_HB128(A_blk, bi_blk):
        Wm = np.zeros((128, 96), np.float32)
        for j in range(2):
            for k in range(4):
                for d in range(12):
                    c = j * 48 + k * 12 + d
                    for dp in range(12):
                        Wm[j * 48 + k * 12 + dp, c] = A_blk[d, dp]
                    if bi_blk is not None:
                        for w in range(4):
                            Wm[96 + j * 16 + k * 4 + w, c] = bi_blk[d]
        return Wm

    out["W_r_H"] = gate_H97(gru_w_hh[0:12], gru_b_ih[0:12] + gru_b_hh[0:12])
    out["W_z_H"] = gate_H97(gru_w_hh[12:24], gru_b_ih[12:24] + gru_b_hh[12:24])
    out["W_hn_H"] = gate_H97(gru_w_hh[24:36], gru_b_hh[24:36])
    out["W_r_HB"] = gate_HB128(A_r, bi_r)
    out["W_z_HB"] = gate_HB128(A_z, bi_z)
    out["W_inn_HB"] = gate_HB128(A_n, bi_n)
    out["b_inn96"] = np.tile(gru_b_ih[24:36], 8).reshape(96, 1).astype(np.float32)

    # LSTM lhsTs. X-part [97, 96]: rows j*48+k*12+d map x (jj = d*4+k),
    # row 96 = folded gate bias. HL-part [96, 96] block-diag W_hh.T.
    gates = {"I": 0, "F": 1, "G": 2, "O": 3}
    wih = {"fw": lstm_w_ih_fw, "bw": lstm_w_ih_bw}
    whh = {"fw": lstm_w_hh_fw, "bw": lstm_w_hh_bw}
    bih = {"fw": lstm_b_ih_fw, "bw": lstm_b_ih_bw}
    bhh = {"fw": lstm_b_hh_fw, "bw": lstm_b_hh_bw}
    for gname, g in gates.items():
        X = np.zeros((97, 96), np.float32)
        Hm = np.zeros((96, 96), np.float32)
        for j, dirn in enumerate(("fw", "bw")):
            wg = wih[dirn][g * 48:(g + 1) * 48, :]
            hg_ = whh[dirn][g * 48:(g + 1) * 48, :]
            for k in range(4):
                for d in range(12):
                    X[j * 48 + k * 12 + d, j * 48:(j + 1) * 48] = wg[:, d * 4 + k]
            X[96, j * 48:(j + 1) * 48] = (bih[dirn][g * 48:(g + 1) * 48]
                                          + bhh[dirn][g * 48:(g + 1) * 48])
            Hm[j * 48:(j + 1) * 48, j * 48:(j + 1) * 48] = hg_.T
        out[f"L_{gname}_X"] = X
        out[f"L_{gname}_HL"] = Hm

    # MLP with folded biases via ones rows.
    We1 = np.zeros((97, 48), np.float32)
    We1[0:48, :] = en1_w[:, 0:48].T
    We1[48:96, :] = en1_w[:, 48:96].T
    We1[96, :] = en1_b
    out["We1"] = We1
    We2 = np.zeros((49, 36), np.float32)
    We2[0:48, :] = en2_w.T
    We2[48, :] = en2_b
    out["We2"] = We2
    We3 = np.zeros((37, 6), np.float32)
    We3[0:36, :] = en3_w.T
    We3[36, :] = en3_b
    out["We3"] = We3
    return out


def prep_inputs(nodes_feature, pos, attmat):
    """feat [S, 48, N] bf16 (k-major (k,d)), att [S, 16, N] bf16,
    arbm [3, 4, 96, N] bf16 (att d-replicated: arbm[g,n,j*48+w*12+d] =
    att[2g+j, n, w]; g=2 j=1 rows zero)."""
    nf = np.concatenate([nodes_feature, pos], axis=-1)       # [N,S,K,12]
    feat = np.ascontiguousarray(nf.transpose(1, 2, 3, 0)).reshape(S, 48, N)
    att = np.ascontiguousarray(attmat.transpose(1, 2, 3, 0)).reshape(S, 16, N)
    att_r = att.reshape(S, 4, 4, N)                          # [s, n, w, N]
    arbm = np.zeros((NGROUPS, 4, 2, 4, 12, N), dtype=BF)
    for g in range(NGROUPS):
        for j in range(2 if g < 2 else 1):
            # [n, w, N] -> replicate d -> [n, w, 12, N]
            arbm[g, :, j] = np.repeat(
                att_r[2 * g + j].astype(BF)[:, :, None, :], 12, axis=2)
    arbm = arbm.reshape(NGROUPS, 4, 96, N)
    return feat.astype(BF), att.astype(BF), arbm


# ----------------------------------------------------------------------------
# device kernel builder
# ----------------------------------------------------------------------------
def split_excess_waits(nc, max_waits=1):
    import concourse.mybir as mybir
    cnt = 0
    for f in nc.m.functions:
        for bb in f.blocks:
            insts = bb.instructions
            new = []
            changed = False
            for inst in insts:
                si = inst.sync_info
                waits = list(si.on_wait) if si and si.on_wait else []
                if len(waits) > max_waits:
                    changed = True
                    k = 0
                    while len(waits) - k > max_waits:
                        chunk = waits[k:k + max_waits]
                        k += max_waits
                        cnt += 1
                        nop = mybir.InstNoOp(name=f"waitsplit-{cnt}", ins=[], outs=[])
                        nop.engine = inst.engine
                        nop.sync_info = mybir.SyncInfo(on_wait=chunk, on_update=[])
                        new.append(nop)
                    inst.sync_info = mybir.SyncInfo(
                        on_wait=waits[k:],
                        on_update=list(si.on_update) if si.on_update else [])
                new.append(inst)
            if changed:
                bb.instructions = new
    return cnt


WEIGHT_SPECS = None  # filled in build_nc


def build_nc():
    import concourse.bass as bass
    import concourse.tile as tile
    from concourse import mybir

    f32 = mybir.dt.float32
    bf16 = mybir.dt.bfloat16
    AF = mybir.ActivationFunctionType
    ALU = mybir.AluOpType
    PSDT = bf16 if BF16_PSUM else f32

    nc = bass.Bass("TRN2")

    feat_d = nc.dram_tensor("feat", [S, 48, NCORE], bf16, kind="ExternalInput")
    att_d = nc.dram_tensor("att", [S, 16, NCORE], bf16, kind="ExternalInput")
    arbm_d = nc.dram_tensor("arbm", [NGROUPS, 4, 96, NCORE], bf16,
                            kind="ExternalInput")

    wspecs = []
    for n in range(4):
        wspecs.append((f"SR{n}", (96, 96), bf16))
        if not ARB_VIA_DMA:
            wspecs.append((f"RA{n}", (32, 96), bf16))
    wspecs += [("W_r_H", (97, 96), bf16), ("W_z_H", (97, 96), bf16),
               ("W_hn_H", (97, 96), bf16),
               ("W_r_HB", (128, 96), bf16), ("W_z_HB", (128, 96), bf16),
               ("W_inn_HB", (128, 96), bf16),
               ("b_inn96", (96, 1), f32)]
    for gname in "IFGO":
        wspecs.append((f"L_{gname}_X", (97, 96), bf16))
        wspecs.append((f"L_{gname}_HL", (96, 96), bf16))
    wspecs += [("We1", (97, 48), bf16), ("We2", (49, 36), bf16),
               ("We3", (37, 6), bf16)]

    wnames = {}
    for nm, shp, dt in wspecs:
        wnames[nm] = nc.dram_tensor(nm, list(shp), dt, kind="ExternalInput")
    out_d = nc.dram_tensor("out", [6, NCORE], f32, kind="ExternalOutput")
    RA_QUAD = {0: 0, 1: 32, 2: 64, 3: 96}

    global WEIGHT_SPECS
    WEIGHT_SPECS = wspecs

    with tile.TileContext(nc) as tc:
        with ExitStack() as ctx:
            wpool = ctx.enter_context(tc.tile_pool(name="weights", bufs=1))
            wt = {}
            for nm, shp, dt in wspecs:
                if nm.startswith("RA"):
                    q = RA_QUAD[int(nm[2])]
                    t = wpool.tile([128, shp[1]], dt, tag=f"w_{nm}")
                    nc.sync.dma_start(t[q:q + 32, :], wnames[nm][:])
                    wt[nm] = t
                else:
                    t = wpool.tile([shp[0], shp[1]], dt, tag=f"w_{nm}")
                    nc.sync.dma_start(t[:], wnames[nm][:])
                    wt[nm] = t

            # persistent per-lane state tiles (ones rows / zero rows set once)
            stp = ctx.enter_context(tc.tile_pool(name="state", bufs=1))
            HG, ATT, ARB, XL, HL, CC, E1, E2 = {}, {}, {}, {}, {}, {}, {}, {}
            for ln in range(NLANES):
                for g in range(NGROUPS):
                    hg = stp.tile([97, PW], bf16, tag=f"hg{ln}_{g}", name=f"hg{ln}_{g}")
                    nc.vector.memset(hg[96:97, :], 1.0)
                    if g == 2:
                        nc.vector.memset(hg[48:96, :], 0.0)
                    HG[(ln, g)] = hg
                    at = stp.tile([32, PW], bf16, tag=f"att{ln}_{g}", name=f"att{ln}_{g}")
                    if g == 2:
                        nc.vector.memset(at[16:32, :], 0.0)
                    ATT[(ln, g)] = at
                    for n in range(4):
                        ab = stp.tile([96, PW], bf16, tag=f"arb{ln}_{g}_{n}",
                                      name=f"arb{ln}_{g}_{n}")
                        if g == 2:
                            nc.vector.memset(ab[48:96, :], 0.0)
                        ARB[(ln, g, n)] = ab
                for kx in range(2):
                    xl = stp.tile([97, PW], bf16, tag=f"xl{ln}_{kx}", name=f"xl{ln}_{kx}")
                    nc.vector.memset(xl[96:97, :], 1.0)
                    XL[(ln, kx)] = xl
                hl = stp.tile([97, PW], bf16, tag=f"hl{ln}", name=f"hl{ln}")
                nc.vector.memset(hl[96:97, :], 1.0)
                HL[ln] = hl
                CC[ln] = stp.tile([96, PW], bf16, tag=f"cc{ln}", name=f"cc{ln}")
                e1 = stp.tile([49, PW], bf16, tag=f"e1{ln}", name=f"e1{ln}")
                nc.vector.memset(e1[48:49, :], 1.0)
                E1[ln] = e1
                E2[ln] = stp.tile([37, PW], bf16, tag=f"e2{ln}", name=f"e2{ln}")

            sbp = ctx.enter_context(tc.tile_pool(name="work", bufs=2))
            psp = ctx.enter_context(tc.tile_pool(name="ps", bufs=2, space="PSUM"))

            def ps_alloc(tag, name, dt=PSDT):
                return psp.tile([96, PW], dt, tag=tag, name=name)

            def emit_pair(ip, ln):
                c0 = ip * PW
                uid = f"p{ip}"
                # ---- loads ----
                for g in range(NGROUPS):
                    hg = HG[(ln, g)]
                    at = ATT[(ln, g)]
                    nc.sync.dma_start(hg[0:48, :], feat_d[2 * g, :, c0:c0 + PW])
                    nc.sync.dma_start(at[0:16, :], att_d[2 * g, :, c0:c0 + PW])
                    if g < 2:
                        nc.sync.dma_start(hg[48:96, :],
                                          feat_d[2 * g + 1, :, c0:c0 + PW])
                        nc.sync.dma_start(at[16:32, :],
                                          att_d[2 * g + 1, :, c0:c0 + PW])
                    # arb replication: row (j,w,d) <- att32 row (j, n, w)
                    for n in range(4):
                        arb = ARB[(ln, g, n)]
                        if ARB_VIA_DMA:
                            rows = 96 if g < 2 else 48
                            nc.sync.dma_start(arb[0:rows, :],
                                              arbm_d[g, n, 0:rows, c0:c0 + PW])

                yield  # loads emitted

                # ---- 2 GRU passes ----
                for pas in range(2):
                    for g in range(NGROUPS):
                        hg = HG[(ln, g)]
                        # Q_n = arb_n * h  (pair-wide)
                        Q = []
                        for n in range(4):
                            q = sbp.tile([96, PW], bf16, tag=f"Q{n}")
                            eng = nc.gpsimd if n == 3 else nc.vector
                            eng.tensor_tensor(q[:], ARB[(ln, g, n)][:],
                                              hg[0:96, :], ALU.mult)
                            Q.append(q)
                        # hbar = sum_n SR_n @ Q_n  (per NB half)
                        ps_hb = ps_alloc("pnh", f"hb_{uid}_{g}_{pas}")
                        for h2 in range(2):
                            sl = ps_hb[:, h2 * NB:(h2 + 1) * NB]
                            for n in range(4):
                                nc.tensor.matmul(
                                    sl, wt[f"SR{n}"][:],
                                    Q[n][:, h2 * NB:(h2 + 1) * NB],
                                    start=(n == 0), stop=(n == 3))
                        hb = sbp.tile([128, PW], bf16, tag="HBs")
                        nc.vector.tensor_copy(hb[0:96, :], ps_hb[:])
                        nc.sync.dma_start(hb[96:128, :], ATT[(ln, g)][:])

                        # gates (quantity-pair psums)
                        ps_r = ps_alloc("prz", f"r_{uid}_{g}_{pas}")
                        ps_z = ps_alloc("prz", f"z_{uid}_{g}_{pas}")
                        ps_in = ps_alloc("pnh", f"in_{uid}_{g}_{pas}")
                        ps_hn = ps_alloc("pnh", f"hn_{uid}_{g}_{pas}")
                        for h2 in range(2):
                            cs = slice(h2 * NB, (h2 + 1) * NB)
                            nc.tensor.matmul(ps_r[:, cs], wt["W_r_H"][:],
                                             hg[0:97, cs], start=True, stop=False)
                            nc.tensor.matmul(ps_r[:, cs], wt["W_r_HB"][:],
                                             hb[0:128, cs], start=False, stop=True)
                            nc.tensor.matmul(ps_z[:, cs], wt["W_z_H"][:],
                                             hg[0:97, cs], start=True, stop=False)
                            nc.tensor.matmul(ps_z[:, cs], wt["W_z_HB"][:],
                                             hb[0:128, cs], start=False, stop=True)
                            nc.tensor.matmul(ps_in[:, cs], wt["W_inn_HB"][:],
                                             hb[0:128, cs], start=True, stop=True)
                            nc.tensor.matmul(ps_hn[:, cs], wt["W_hn_H"][:],
                                             hg[0:97, cs], start=True, stop=True)

                        yield  # gates emitted; let other lane feed PE
                        sr = sbp.tile([96, PW], bf16, tag="SR_")
                        sz = sbp.tile([96, PW], bf16, tag="SZ_")
                        nc.scalar.activation(sr[:], ps_r[:], AF.Sigmoid)
                        nc.scalar.activation(sz[:], ps_z[:], AF.Sigmoid)
                        t1 = sbp.tile([96, PW], bf16, tag="t1")
                        nc.vector.tensor_tensor(t1[:], ps_hn[:], sr[:], ALU.mult)
                        u = sbp.tile([96, PW], bf16, tag="u")
                        nc.vector.tensor_tensor(u[:], ps_in[:], t1[:], ALU.add)
                        tn = sbp.tile([96, PW], bf16, tag="tn")
                        nc.scalar.activation(tn[:], u[:], AF.Tanh,
                                             bias=wt["b_inn96"][:, 0:1])
                        v = sbp.tile([96, PW], bf16, tag="v")
                        nc.vector.tensor_tensor(v[:], hg[0:96, :], tn[:],
                                                ALU.subtract)
                        w2 = sbp.tile([96, PW], bf16, tag="w2")
                        nc.vector.tensor_tensor(w2[:], sz[:], v[:], ALU.mult)
                        rows = slice(0, 48) if g == 2 else slice(0, 96)
                        nc.vector.tensor_tensor(hg[rows, :], tn[rows, :],
                                                w2[rows, :], ALU.add)
                        yield  # (pass, group) emitted

                # ---- BiLSTM ----
                hl = HL[ln]
                cc = CC[ln]
                for t in range(S):
                    sf_, sb_ = t, 4 - t
                    xl = XL[(ln, t % 2)]
                    nc.sync.dma_start(
                        xl[0:48, :],
                        HG[(ln, sf_ // 2)][(sf_ % 2) * 48:(sf_ % 2) * 48 + 48, :])
                    nc.sync.dma_start(
                        xl[48:96, :],
                        HG[(ln, sb_ // 2)][(sb_ % 2) * 48:(sb_ % 2) * 48 + 48, :])
                    ps_g = {}
                    for gname, tag in (("I", "prz"), ("F", "prz"),
                                       ("G", "pnh"), ("O", "pnh")):
                        if t == 0 and gname == "F":
                            continue
                        ps = ps_alloc(tag, f"L{gname}_{uid}_{t}")
                        ps_g[gname] = ps
                        for h2 in range(2):
                            cs = slice(h2 * NB, (h2 + 1) * NB)
                            nc.tensor.matmul(ps[:, cs], wt[f"L_{gname}_X"][:],
                                             xl[0:97, cs],
                                             start=True, stop=(t == 0))
                            if t > 0:
                                nc.tensor.matmul(ps[:, cs],
                                                 wt[f"L_{gname}_HL"][:],
                                                 hl[0:96, cs],
                                                 start=False, stop=True)
                    si = sbp.tile([96, PW], bf16, tag="si")
                    nc.scalar.activation(si[:], ps_g["I"][:], AF.Sigmoid)
                    tg = sbp.tile([96, PW], bf16, tag="tg")
                    nc.scalar.activation(tg[:], ps_g["G"][:], AF.Tanh)
                    t1l = sbp.tile([96, PW], bf16, tag="t1l")
                    nc.vector.tensor_tensor(t1l[:], si[:], tg[:], ALU.mult)
                    if t == 0:
                        nc.vector.tensor_copy(cc[:], t1l[:])
                    else:
                        sf2 = sbp.tile([96, PW], bf16, tag="sf2")
                        nc.scalar.activation(sf2[:], ps_g["F"][:], AF.Sigmoid)
                        t2l = sbp.tile([96, PW], bf16, tag="t2l")
                        nc.gpsimd.tensor_tensor(t2l[:], sf2[:], cc[:], ALU.mult)
                        nc.vector.tensor_tensor(cc[:], t1l[:], t2l[:], ALU.add)
                    tc2 = sbp.tile([96, PW], bf16, tag="tc2")
                    nc.scalar.activation(tc2[:], cc[:], AF.Tanh)
                    so = sbp.tile([96, PW], bf16, tag="so")
                    nc.scalar.activation(so[:], ps_g["O"][:], AF.Sigmoid)
                    nc.vector.tensor_tensor(hl[0:96, :], so[:], tc2[:], ALU.mult)
                    yield  # LSTM step emitted

                # ---- MLP ----
                psE = ps_alloc("prz", f"psE_{uid}", dt=f32)
                for h2 in range(2):
                    cs = slice(h2 * NB, (h2 + 1) * NB)
                    nc.tensor.matmul(psE[0:48, cs], wt["We1"][:], hl[0:97, cs],
                                     start=True, stop=True)
                e1 = E1[ln]
                nc.scalar.activation(e1[0:48, :], psE[0:48, :], AF.Relu)
                psE2 = ps_alloc("pnh", f"psE2_{uid}", dt=f32)
                for h2 in range(2):
                    cs = slice(h2 * NB, (h2 + 1) * NB)
                    nc.tensor.matmul(psE2[0:36, cs], wt["We2"][:], e1[0:49, cs],
                                     start=True, stop=True)
                e2 = E2[ln]
                nc.scalar.activation(e2[0:36, :], psE2[0:36, :], AF.Relu)
                psE3 = ps_alloc("prz", f"psE3_{uid}", dt=f32)
                for h2 in range(2):
                    cs = slice(h2 * NB, (h2 + 1) * NB)
                    nc.tensor.matmul(psE3[0:6, cs], wt["We3"][:], e2[0:37, cs],
                                     start=True, stop=True)
                o = sbp.tile([6, PW], f32, tag="o")
                nc.vector.tensor_copy(o[:], psE3[0:6, :])
                nc.sync.dma_start(out_d[:, c0:c0 + PW], o[:])

            for kp in range(0, NPAIRS, NLANES):
                gens = [emit_pair(kp + i, i) for i in range(NLANES)]
                done = [False] * NLANES
                while not all(done):
                    for i, gg in enumerate(gens):
                        if not done[i]:
                            try:
                                next(gg)
                            except StopIteration:
                                done[i] = True

    split_excess_waits(nc)
    return nc


_NC_CACHE = None
TRACE = False
LAST_EXEC_NS = None


def kernel(nodes_feature, pos, attmat, **w):
    global _NC_CACHE, LAST_EXEC_NS
    from concourse.bass_utils import run_bass_kernel_spmd
    import concourse.mybir as mybir

    feat, att, arbm = prep_inputs(nodes_feature, pos, attmat)
    wts = build_weights(**w)

    if _NC_CACHE is None:
        _NC_CACHE = build_nc()
    nc = _NC_CACHE

    in_maps = []
    for c in range(NCORES):
        m = {"feat": np.ascontiguousarray(feat[:, :, c * NCORE:(c + 1) * NCORE]),
             "att": np.ascontiguousarray(att[:, :, c * NCORE:(c + 1) * NCORE]),
             "arbm": np.ascontiguousarray(arbm[:, :, :, c * NCORE:(c + 1) * NCORE])}
        for nm, shp, dt in WEIGHT_SPECS:
            m[nm] = wts[nm].astype(BF) if dt == mybir.dt.bfloat16 else wts[nm].astype(np.float32)
        in_maps.append(m)

    res = run_bass_kernel_spmd(nc, in_maps, core_ids=list(range(NCORES)),
                               trace=TRACE)
    LAST_EXEC_NS = res.exec_time_ns
    outs = [res.results[c]["out"] for c in range(NCORES)]     # [6, NCORE] each
    full = np.concatenate(outs, axis=1)                        # [6, N]
    return np.ascontiguousarray(full.T).astype(np.float32)     # [N, 6]
